# revision 12
# baseline (speedup 1.0000x reference)
"""TRN2 Bass kernel for nn_AutoEncoder_14542759264279 (scatter_memory).

Problem (per sample b of 8): scatter-add 500k values into a 128^3 grid by
int coordinates, then total-variation (sum |adjacent diff|) and smoothness
MSE (sum diff^2) losses over the grid, each normalized. Output (2, 8) f32.

Sharding: data-parallel over the batch axis - core b handles sample b
entirely (its own scatter + losses), no cross-core traffic.

Device algorithm per core (one-hot matmul scatter; no indirect DMA):
  - host bins points by x0 into 128 slabs (pure permutation + zero pad),
    packing per-slab points into 128-point chunks: column j of the
    [128, C] inputs X1/X2/V holds chunk j's points, one per partition.
  - per chunk the DVE builds two bf16 one-hot matrices with single fused
    tensor_scalar ops: Bv = (iota==x1)*v and Cm = (iota==x2); the PE
    matmul Bv^T @ Cm scatter-adds the chunk into the slab's [x1, x2]
    PSUM accumulator (chunks of one slab form one PSUM accumulation
    chain; slabs rotate over PSUM banks). Slabs copy to an SBUF-resident
    [x1, x0, x2] grid via the Act engine.
  - losses: x0/x2 diffs are free-axis DVE subtracts over big 3D tiles;
    x1 (partition) diffs via a constant shift-matrix matmul; |.| and ^2
    reductions run on the Act engine with accum_out, reading PSUM
    directly for the x1 diffs.

Self-contained: hardcodes all shapes; no file reads.
"""
import contextlib
import ctypes
import math
import sys
import types

import numpy as np

P = 128
XS = 128
B = 8
M = 500_000
TV_NORM = float(XS * XS * XS)
MSE_NORM = float(2 * XS * XS - 2 * XS)

_SO_PATH = "/opt/axon/libaxon_pjrt.so"


def _install_ntff_hook():
    """Provide antenv.axon_hooks (NTFF profile hook) if missing."""
    if "antenv.axon_hooks" in sys.modules:
        return
    try:
        import antenv
    except ImportError:
        return

    def _make_hook():
        try:
            lib = ctypes.CDLL(_SO_PATH)
        except OSError:
            return None
        if not hasattr(lib, "axon_start_nrt_profile"):
            return None
        lib.axon_start_nrt_profile.argtypes = [
            ctypes.POINTER(ctypes.c_int64),
            ctypes.c_size_t,
        ]
        lib.axon_start_nrt_profile.restype = ctypes.c_int64
        lib.axon_stop_nrt_profile.argtypes = [ctypes.c_char_p]
        lib.axon_stop_nrt_profile.restype = ctypes.c_int64

        @contextlib.contextmanager
        def _hook(output_dir, device_ids):
            import jax

            jax.devices()
            if device_ids:
                ids = (ctypes.c_int64 * len(device_ids))(*device_ids)
                rc = lib.axon_start_nrt_profile(ids, len(device_ids))
            else:
                rc = lib.axon_start_nrt_profile(None, 0)
            if rc != 0:
                raise RuntimeError(f"axon_start_nrt_profile rc={rc}")
            try:
                yield
            finally:
                n = lib.axon_stop_nrt_profile(str(output_dir).encode())
                print(f"ntff profile: {n} file(s) in {output_dir}", file=sys.stderr)

        return _hook

    mod = types.ModuleType("antenv.axon_hooks")
    mod._hook = _make_hook()
    mod.get_axon_ntff_profile_hook = lambda: mod._hook

    def _set(h):
        mod._hook = h

    mod.set_axon_ntff_profile_hook = _set
    sys.modules["antenv.axon_hooks"] = mod
    antenv.axon_hooks = mod


def _split_waits(nc, mybir):
    """walrus here allows only 1 sem wait per instruction; hoist extras
    onto preceding same-engine NoOps."""
    n = 0
    for f in nc.m.functions:
        for bb in f.blocks:
            il = bb.instructions
            i = 0
            while i < len(il):
                inst = il[i]
                si = inst.sync_info
                if si is not None and len(si.on_wait) > 1:
                    waits = list(si.on_wait)
                    si.on_wait = waits[:1]
                    pre = []
                    for w in waits[1:]:
                        nop = mybir.InstNoOp(name=f"I-waitsplit-{n}", ins=[], outs=[])
                        n += 1
                        nop.engine = inst.engine
                        nop.sync_info = mybir.SyncInfo(on_wait=[w], on_update=[])
                        pre.append(nop)
                    il[i:i] = pre
                    i += len(pre)
                i += 1
    return n


def _patch_tile_drain(tile, bass_rust, mybir):
    """Split the tail-drain waits (same 1-wait-per-instruction limit)."""

    def _drain_and_barrier(self, tick_clock, wait_clock):
        drain_inst = self.nc.sync.drain()
        wait_clock.add_sem_waits(
            drain_inst.ins, bass_rust.ScopedClock({None: tick_clock.global_clock})
        )
        si = drain_inst.ins.sync_info
        waits = list(si.on_wait) if si is not None else []
        if len(waits) > 1:
            si.on_wait = waits[:1]
            for i in range(1, len(waits)):
                extra = self.nc.sync.drain()
                esi = extra.ins.sync_info
                if esi is None:
                    extra.ins.sync_info = mybir.SyncInfo(
                        on_wait=[waits[i]], on_update=[]
                    )
                else:
                    esi.on_wait = [waits[i]]
        self.nc.all_engine_barrier()
        assert self.sems is not None
        popped = self.nc._tile_sem_poison_stack.pop()
        assert popped is self._sem_poison
        sems = sorted(
            s.num if hasattr(s, "num") else s
            for s in self.sems.allocated().values()
        )
        for i in range(0, len(sems), 4):
            self.nc.clear_and_free_semaphores(sems[i : i + 4])
        self.nc.all_engine_barrier()

    tile.TileContext._drain_and_barrier = _drain_and_barrier


def build_program(T, ocnk, split_num, split_den, sub_q7):
    """T: per-slab diag round counts; ocnk: per-slab overflow chunk counts.

    Diag scheme: points binned by (x0, x1); round j of slab a holds the
    j-th point of each (a, q) bin at partition q (=x1), so the matmul
    stationary is a fixed identity and ONE fused tensor_scalar builds the
    value-scaled x2 one-hot. Bins deeper than T[a] overflow into generic
    two-one-hot chunks (the v1 path) appended to the same PSUM chain.
    """
    import os
    import concourse.bass as bass
    import concourse.mybir as mybir
    import concourse.tile as tile
    import bass_rust

    _patch_tile_drain(tile, bass_rust, mybir)

    f32 = mybir.dt.float32
    bf16 = mybir.dt.bfloat16
    Alu = mybir.AluOpType
    Act = mybir.ActivationFunctionType

    R = sum(T)
    ro = [0] * 128
    for a in range(1, 128):
        ro[a] = ro[a - 1] + T[a - 1]
    CO = max(1, sum(ocnk))
    oofs = [0] * 128
    for a in range(1, 128):
        oofs[a] = oofs[a - 1] + ocnk[a - 1]

    nc = bass.Bass("TRN2", target_bir_lowering=False, debug=False)
    x2r_d = nc.dram_tensor("x2r", [P, R], f32, kind="ExternalInput")
    vr_d = nc.dram_tensor("vr", [P, R], f32, kind="ExternalInput")
    x1o_d = nc.dram_tensor("x1o", [P, CO], f32, kind="ExternalInput")
    x2o_d = nc.dram_tensor("x2o", [P, CO], f32, kind="ExternalInput")
    vo_d = nc.dram_tensor("vo", [P, CO], f32, kind="ExternalInput")
    iota_d = nc.dram_tensor("iota", [P, P], bf16, kind="ExternalInput")
    ident_d = nc.dram_tensor("ident", [P, P], bf16, kind="ExternalInput")
    sdiff_d = nc.dram_tensor("sdiff", [P, P], f32, kind="ExternalInput")
    out_d = nc.dram_tensor("out", [1, 2], f32, kind="ExternalOutput")

    NACC = 48  # 32 d2-group slots + 8 d1 + 8 d3

    with tile.TileContext(nc) as tc:
        with tc.tile_pool(name="setup", bufs=1) as sp:
            iota_t = sp.tile([P, P], bf16)
            ident_t = sp.tile([P, P], bf16)
            sdiff_t = sp.tile([P, P], f32)
            nc.sync.dma_start(out=iota_t[:], in_=iota_d.ap()[:])
            nc.sync.dma_start(out=ident_t[:], in_=ident_d.ap()[:])
            nc.sync.dma_start(out=sdiff_t[:], in_=sdiff_d.ap()[:])
            x2r_t = sp.tile([P, R], f32)
            vr_t = sp.tile([P, R], f32)
            nch = 8
            for c0 in range(0, R, -(-R // nch)):
                c1 = min(R, c0 + -(-R // nch))
                nc.sync.dma_start(out=x2r_t[:, c0:c1], in_=x2r_d.ap()[:, c0:c1])
                nc.sync.dma_start(out=vr_t[:, c0:c1], in_=vr_d.ap()[:, c0:c1])
            x1o_t = sp.tile([P, CO], f32)
            x2o_t = sp.tile([P, CO], f32)
            vo_t = sp.tile([P, CO], f32)
            nc.sync.dma_start(out=x1o_t[:], in_=x1o_d.ap()[:])
            nc.sync.dma_start(out=x2o_t[:], in_=x2o_d.ap()[:])
            nc.sync.dma_start(out=vo_t[:], in_=vo_d.ap()[:])

            G = sp.tile([P, 128, XS], f32)  # [x1, x0, x2] grid
            accT = sp.tile([P, NACC], f32)
            accM = sp.tile([P, NACC], f32)
            nc.vector.memset(accT[:], 0.0)
            nc.vector.memset(accM[:], 0.0)

            with tc.tile_pool(name="oh", bufs=12) as oh, \
                 tc.tile_pool(name="ps", bufs=4, space="PSUM") as ps, \
                 tc.tile_pool(name="d2ps", bufs=2, space="PSUM") as d2ps, \
                 tc.tile_pool(name="fin", bufs=1, space="PSUM") as fin, \
                 tc.tile_pool(name="ls", bufs=2) as lsp:

                slot = [0]

                def emit_act_reduce(src_ap, nfd, wfd):
                    """abs-sum and square-sum of src ([P, nfd, wfd]) into the
                    next acc slot."""
                    n = slot[0]
                    dump = lsp.tile([P, 16, XS], f32, tag="dump")
                    nc.scalar.activation(
                        out=dump[:, :nfd, :wfd], in_=src_ap, func=Act.Abs,
                        accum_out=accT[:, n : n + 1],
                    )
                    dump2 = lsp.tile([P, 16, XS], f32, tag="dump2")
                    nc.scalar.activation(
                        out=dump2[:, :nfd, :wfd], in_=src_ap, func=Act.Square,
                        accum_out=accM[:, n : n + 1],
                    )
                    slot[0] += 1

                # ---- scatter: one PSUM accumulation chain per x0-slab ----
                ts_idx = 0
                for a in range(128):
                    nmm = T[a] + ocnk[a]
                    mi = 0
                    pt = ps.tile([P, 512], f32, space="PSUM", tag="scat")
                    for j in range(T[a]):
                        col = ro[a] + j
                        dm = oh.tile([P, P], bf16, tag="dm")
                        ts_idx += 1
                        if split_den and (ts_idx % split_den) < split_num:
                            # is_eq on DVE (2x mode), value-scale on Act
                            d01 = oh.tile([P, P], bf16, tag="d01")
                            nc.vector.tensor_scalar(
                                out=d01[:], in0=iota_t[:],
                                scalar1=x2r_t[:, col : col + 1],
                                scalar2=None, op0=Alu.is_equal,
                            )
                            nc.scalar.activation(
                                out=dm[:], in_=d01[:],
                                func=Act.Copy,
                                scale=vr_t[:, col : col + 1],
                            )
                        else:
                            nc.vector.tensor_scalar(
                                out=dm[:], in0=iota_t[:],
                                scalar1=x2r_t[:, col : col + 1],
                                scalar2=vr_t[:, col : col + 1],
                                op0=Alu.is_equal, op1=Alu.mult,
                            )
                        nc.tensor.matmul(
                            out=pt[:, :P], lhsT=ident_t[:], rhs=dm[:],
                            start=(mi == 0), stop=(mi == nmm - 1),
                        )
                        mi += 1
                    for j in range(ocnk[a]):
                        col = oofs[a] + j
                        bv = oh.tile([P, P], bf16, tag="bv")
                        nc.vector.tensor_scalar(
                            out=bv[:], in0=iota_t[:],
                            scalar1=x1o_t[:, col : col + 1],
                            scalar2=vo_t[:, col : col + 1],
                            op0=Alu.is_equal, op1=Alu.mult,
                        )
                        cm = oh.tile([P, P], bf16, tag="cm")
                        nc.vector.tensor_scalar(
                            out=cm[:], in0=iota_t[:],
                            scalar1=x2o_t[:, col : col + 1],
                            scalar2=None, op0=Alu.is_equal,
                        )
                        nc.tensor.matmul(
                            out=pt[:, :P], lhsT=bv[:], rhs=cm[:],
                            start=(mi == 0), stop=(mi == nmm - 1),
                        )
                        mi += 1
                    nc.scalar.copy(out=G[:, a, :], in_=pt[:, :P])

                    # d2 (x1/partition diffs) for each finished 4-slab group
                    if a % 4 == 3:
                        q = a // 4
                        dp = d2ps.tile([P, 4, XS], f32, space="PSUM", tag="d2")
                        nc.tensor.matmul(
                            out=dp[:], lhsT=sdiff_t[:],
                            rhs=G[:, 4 * q : 4 * q + 4, :],
                            start=True, stop=True,
                        )
                        emit_act_reduce(dp[:], 4, XS)

                    # d3 (x2 diffs) for each finished 16-slab group
                    sub_eng = nc.gpsimd if sub_q7 else nc.vector
                    if a % 16 == 15:
                        g = a // 16
                        lo = 16 * g
                        d = lsp.tile([P, 16, XS], f32, tag="dsub")
                        sub_eng.tensor_tensor(
                            out=d[:, :, : XS - 1],
                            in0=G[:, lo : lo + 16, 1:],
                            in1=G[:, lo : lo + 16, : XS - 1],
                            op=Alu.subtract,
                        )
                        emit_act_reduce(d[:, :, : XS - 1], 16, XS - 1)
                    # d1 (x0 diffs): group g needs slabs up to 16g+16
                    if (a % 16 == 0 and a > 0) or a == 127:
                        g = (a - 1) // 16 if a < 127 else 7
                        lo = 16 * g
                        npair = 16 if g < 7 else 15
                        d = lsp.tile([P, 16, XS], f32, tag="dsub")
                        sub_eng.tensor_tensor(
                            out=d[:, :npair, :],
                            in0=G[:, lo + 1 : lo + 1 + npair, :],
                            in1=G[:, lo : lo + npair, :],
                            op=Alu.subtract,
                        )
                        emit_act_reduce(d[:, :npair, :], npair, XS)

                # ---- optional DVE-variant probe (diagnostic only) ----
                if int(os.environ.get("TRNK_PROBE", "0")):
                    with tc.tile_pool(name="probe", bufs=4) as pp:
                        iota_f32 = sp.tile([P, P], f32)
                        nc.scalar.copy(out=iota_f32[:], in_=iota_t[:])
                        s1 = x2r_t[:, 0:1]
                        s2 = vr_t[:, 0:1]
                        NREP = 256
                        for i in range(NREP):
                            o = pp.tile([P, P], bf16, tag="p1")
                            nc.vector.tensor_scalar(
                                out=o[:], in0=iota_t[:], scalar1=s1, scalar2=s2,
                                op0=Alu.is_equal, op1=Alu.mult)
                        for i in range(NREP):
                            o = pp.tile([P, P], bf16, tag="p2")
                            nc.vector.tensor_scalar(
                                out=o[:], in0=iota_t[:], scalar1=s1, scalar2=None,
                                op0=Alu.is_equal)
                        for i in range(NREP):
                            o = pp.tile([P, P], bf16, tag="p3")
                            nc.vector.tensor_scalar(
                                out=o[:], in0=iota_t[:], scalar1=s2, scalar2=None,
                                op0=Alu.mult)
                        for i in range(NREP):
                            o = pp.tile([P, P], bf16, tag="p4")
                            nc.vector.tensor_scalar(
                                out=o[:], in0=iota_t[:], scalar1=3.0, scalar2=None,
                                op0=Alu.mult)
                        for i in range(NREP):
                            o = pp.tile([P, P], f32, tag="p5")
                            nc.vector.tensor_scalar(
                                out=o[:], in0=iota_t[:], scalar1=s1, scalar2=s2,
                                op0=Alu.is_equal, op1=Alu.mult)
                        for i in range(NREP):
                            o = pp.tile([P, P], bf16, tag="p6")
                            nc.vector.tensor_scalar(
                                out=o[:], in0=iota_f32[:], scalar1=s1, scalar2=s2,
                                op0=Alu.is_equal, op1=Alu.mult)
                        for i in range(NREP):
                            o = pp.tile([P, P], bf16, tag="p7")
                            nc.vector.tensor_scalar(
                                out=o[:], in0=iota_t[:], scalar1=s1, scalar2=3.0,
                                op0=Alu.is_equal, op1=Alu.mult)
                        for i in range(NREP):
                            o = pp.tile([P, P], bf16, tag="p8")
                            nc.vector.tensor_tensor(
                                out=o[:], in0=iota_t[:], in1=ident_t[:],
                                op=Alu.is_equal)
                        for i in range(NREP):
                            o = pp.tile([P, 2, P], bf16, tag="p9")
                            nc.vector.tensor_scalar(
                                out=o[:], in0=G[:, 0:2, :], scalar1=s1, scalar2=s2,
                                op0=Alu.is_equal, op1=Alu.mult)
                        for i in range(NREP):
                            o = pp.tile([P, P], bf16, tag="p10")
                            nc.scalar.activation(
                                out=o[:], in_=iota_t[:], func=Act.Copy, scale=s2)

                # ---- final: reduce slots, cross-partition sum, scale ----
                tvcol = sp.tile([P, 1], f32)
                msecol = sp.tile([P, 1], f32)
                nc.vector.tensor_reduce(
                    out=tvcol[:], in_=accT[:], axis=mybir.AxisListType.X,
                    op=Alu.add,
                )
                nc.vector.tensor_reduce(
                    out=msecol[:], in_=accM[:], axis=mybir.AxisListType.X,
                    op=Alu.add,
                )
                ones = sp.tile([P, 1], f32)
                nc.vector.memset(ones[:], 1.0)
                tv_ps = fin.tile([P, 512], f32, space="PSUM", tag="fint")
                nc.tensor.matmul(out=tv_ps[0:1, 0:1], lhsT=tvcol[:], rhs=ones[:],
                                 start=True, stop=True)
                mse_ps = fin.tile([P, 512], f32, space="PSUM", tag="finm")
                nc.tensor.matmul(out=mse_ps[0:1, 0:1], lhsT=msecol[:], rhs=ones[:],
                                 start=True, stop=True)
                res = sp.tile([1, 2], f32)
                nc.scalar.mul(out=res[:, 0:1], in_=tv_ps[0:1, 0:1], mul=1.0 / TV_NORM)
                nc.scalar.mul(out=res[:, 1:2], in_=mse_ps[0:1, 0:1], mul=1.0 / MSE_NORM)
                nc.sync.dma_start(out=out_d.ap()[:], in_=res[:])

    _split_waits(nc, mybir)
    return nc


_PROG_CACHE = {}


def _get_program(T, ocnk, split_num, split_den, sub_q7):
    key = (tuple(T), tuple(ocnk), split_num, split_den, sub_q7)
    if key not in _PROG_CACHE:
        _PROG_CACHE[key] = build_program(
            tuple(T), tuple(ocnk), split_num, split_den, sub_q7
        )
    return _PROG_CACHE[key]


def _pack_inputs(indices, values, tcap):
    """Bin points by (x0, x1); rounds place the j-th point of bin (a, q)
    at partition q; bins deeper than T[a] overflow into v1-style chunks."""
    import ml_dtypes

    i0 = indices[:, :, 0].astype(np.int64)
    i1 = indices[:, :, 1].astype(np.int64)
    i2 = indices[:, :, 2]
    key = i0 * P + i1
    cnt2 = np.zeros((B, 128 * P), np.int64)
    for b in range(B):
        cnt2[b] = np.bincount(key[b], minlength=128 * P)
    mx2 = cnt2.max(axis=0).reshape(128, P)  # worst-core per-(a,q) bin size
    T = np.maximum(1, np.minimum(mx2.max(axis=1), tcap)).astype(np.int64)
    ro = np.zeros(128, np.int64)
    ro[1:] = np.cumsum(T)[:-1]
    R = int(T.sum())
    ovc = np.maximum(0, cnt2.reshape(B, 128, P) - T[None, :, None]).sum(axis=2)
    ocnk = (-(-ovc.max(axis=0) // P)).astype(np.int64)  # [128]
    oofs = np.zeros(128, np.int64)
    oofs[1:] = np.cumsum(ocnk)[:-1]
    CO = max(1, int(ocnk.sum()))

    iota_in = np.broadcast_to(np.arange(P, dtype=np.float32), (P, P)).astype(
        ml_dtypes.bfloat16
    )
    ident_in = np.eye(P, dtype=np.float32).astype(ml_dtypes.bfloat16)

    in_maps = []
    for b in range(B):
        order = np.argsort(key[b], kind="stable")
        ks = key[b][order]
        a_s = ks // P
        q_s = ks % P
        i2s = i2[b][order].astype(np.float32)
        vs = values[b][order]
        starts2 = np.zeros(128 * P + 1, np.int64)
        starts2[1:] = np.cumsum(cnt2[b])
        rank = np.arange(M, dtype=np.int64) - starts2[ks]
        dmask = rank < T[a_s]
        X2R = np.zeros((P, R), np.float32)
        VR = np.zeros((P, R), np.float32)
        X2R[q_s[dmask], ro[a_s[dmask]] + rank[dmask]] = i2s[dmask]
        VR[q_s[dmask], ro[a_s[dmask]] + rank[dmask]] = vs[dmask]
        X1O = np.zeros((P, CO), np.float32)
        X2O = np.zeros((P, CO), np.float32)
        VO = np.zeros((P, CO), np.float32)
        ov = ~dmask
        if ov.any():
            cs = np.cumsum(ov)
            slab_start = starts2[a_s * P]  # first point idx of this slab
            cs0 = np.zeros(M + 1, np.int64)
            cs0[1:] = cs
            ovrank = cs - 1 - cs0[slab_start]
            col = oofs[a_s[ov]] + ovrank[ov] // P
            part = ovrank[ov] % P
            X1O[part, col] = q_s[ov].astype(np.float32)
            X2O[part, col] = i2s[ov]
            VO[part, col] = vs[ov]
        in_maps.append(
            {
                "x2r": X2R,
                "vr": VR,
                "x1o": X1O,
                "x2o": X2O,
                "vo": VO,
                "iota": iota_in,
                "ident": ident_in,
                "sdiff": _sdiff_mat(),
            }
        )
    return T, ocnk, in_maps


def _sdiff_mat():
    # sdiff[x1, m] = +1 if x1==m+1 else -1 if x1==m (column 127 zeroed)
    sd = np.zeros((P, P), np.float32)
    for m in range(P - 1):
        sd[m + 1, m] = 1.0
        sd[m, m] = -1.0
    return sd


def kernel(indices, values, xsize):
    sys.path.insert(0, "/opt/trn_rl_repo")
    _install_ntff_hook()
    from concourse import bass_utils

    indices = np.asarray(indices, dtype=np.int32)
    values = np.asarray(values, dtype=np.float32)
    assert int(xsize) == XS
    assert indices.shape == (B, M, 3) and values.shape == (B, M)

    import os

    tcap = int(os.environ.get("TRNK_TCAP", "32"))
    split_num = int(os.environ.get("TRNK_SPLIT_NUM", "0"))
    split_den = int(os.environ.get("TRNK_SPLIT_DEN", "0"))
    sub_q7 = int(os.environ.get("TRNK_SUB_Q7", "1"))
    T, ocnk, in_maps = _pack_inputs(indices, values, tcap)
    nc = _get_program(T, ocnk, split_num, split_den, sub_q7)

    trace = bool(os.environ.get("TRNK_TRACE"))
    res = bass_utils.run_bass_kernel_spmd(
        nc, in_maps, core_ids=list(range(B)), trace=trace
    )
    if trace and res.exec_time_ns is not None:
        print(f"HW exec time: {res.exec_time_ns} ns")
    tv = np.array([res.results[b]["out"][0, 0] for b in range(B)], np.float32)
    mse = np.array([res.results[b]["out"][0, 1] for b in range(B)], np.float32)
    return np.stack([tv, mse]).astype(np.float32)


if __name__ == "__main__":
    rng = np.random.default_rng(0)
    idx = rng.integers(0, XS, (B, M, 3), dtype=np.int32)
    val = rng.standard_normal((B, M), dtype=np.float32)
    out = kernel(idx, val, XS)
    print(out)


# revision 13
# speedup vs baseline: 1.1589x; 1.1589x over previous
"""TRN2 Bass kernel for nn_AutoEncoder_14542759264279 (scatter_memory).

Problem (per sample b of 8): scatter-add 500k values into a 128^3 grid by
int coordinates, then total-variation (sum |adjacent diff|) and smoothness
MSE (sum diff^2) losses over the grid, each normalized. Output (2, 8) f32.

Sharding: data-parallel over the batch axis - core b handles sample b
entirely (its own scatter + losses), no cross-core traffic.

Device algorithm per core (one-hot matmul scatter; no indirect DMA):
  - host bins points by x0 into 128 slabs (pure permutation + zero pad),
    packing per-slab points into 128-point chunks: column j of the
    [128, C] inputs X1/X2/V holds chunk j's points, one per partition.
  - per chunk the DVE builds two bf16 one-hot matrices with single fused
    tensor_scalar ops: Bv = (iota==x1)*v and Cm = (iota==x2); the PE
    matmul Bv^T @ Cm scatter-adds the chunk into the slab's [x1, x2]
    PSUM accumulator (chunks of one slab form one PSUM accumulation
    chain; slabs rotate over PSUM banks). Slabs copy to an SBUF-resident
    [x1, x0, x2] grid via the Act engine.
  - losses: x0/x2 diffs are free-axis DVE subtracts over big 3D tiles;
    x1 (partition) diffs via a constant shift-matrix matmul; |.| and ^2
    reductions run on the Act engine with accum_out, reading PSUM
    directly for the x1 diffs.

Self-contained: hardcodes all shapes; no file reads.
"""
import contextlib
import ctypes
import math
import sys
import types

import numpy as np

P = 128
XS = 128
B = 8
M = 500_000
TV_NORM = float(XS * XS * XS)
MSE_NORM = float(2 * XS * XS - 2 * XS)

_SO_PATH = "/opt/axon/libaxon_pjrt.so"


def _install_ntff_hook():
    """Provide antenv.axon_hooks (NTFF profile hook) if missing."""
    if "antenv.axon_hooks" in sys.modules:
        return
    try:
        import antenv
    except ImportError:
        return

    def _make_hook():
        try:
            lib = ctypes.CDLL(_SO_PATH)
        except OSError:
            return None
        if not hasattr(lib, "axon_start_nrt_profile"):
            return None
        lib.axon_start_nrt_profile.argtypes = [
            ctypes.POINTER(ctypes.c_int64),
            ctypes.c_size_t,
        ]
        lib.axon_start_nrt_profile.restype = ctypes.c_int64
        lib.axon_stop_nrt_profile.argtypes = [ctypes.c_char_p]
        lib.axon_stop_nrt_profile.restype = ctypes.c_int64

        @contextlib.contextmanager
        def _hook(output_dir, device_ids):
            import jax

            jax.devices()
            if device_ids:
                ids = (ctypes.c_int64 * len(device_ids))(*device_ids)
                rc = lib.axon_start_nrt_profile(ids, len(device_ids))
            else:
                rc = lib.axon_start_nrt_profile(None, 0)
            if rc != 0:
                raise RuntimeError(f"axon_start_nrt_profile rc={rc}")
            try:
                yield
            finally:
                n = lib.axon_stop_nrt_profile(str(output_dir).encode())
                print(f"ntff profile: {n} file(s) in {output_dir}", file=sys.stderr)

        return _hook

    mod = types.ModuleType("antenv.axon_hooks")
    mod._hook = _make_hook()
    mod.get_axon_ntff_profile_hook = lambda: mod._hook

    def _set(h):
        mod._hook = h

    mod.set_axon_ntff_profile_hook = _set
    sys.modules["antenv.axon_hooks"] = mod
    antenv.axon_hooks = mod


def _split_waits(nc, mybir):
    """walrus here allows only 1 sem wait per instruction; hoist extras
    onto preceding same-engine NoOps."""
    n = 0
    for f in nc.m.functions:
        for bb in f.blocks:
            il = bb.instructions
            i = 0
            while i < len(il):
                inst = il[i]
                si = inst.sync_info
                if si is not None and len(si.on_wait) > 1:
                    waits = list(si.on_wait)
                    si.on_wait = waits[:1]
                    pre = []
                    for w in waits[1:]:
                        nop = mybir.InstNoOp(name=f"I-waitsplit-{n}", ins=[], outs=[])
                        n += 1
                        nop.engine = inst.engine
                        nop.sync_info = mybir.SyncInfo(on_wait=[w], on_update=[])
                        pre.append(nop)
                    il[i:i] = pre
                    i += len(pre)
                i += 1
    return n


def _patch_tile_drain(tile, bass_rust, mybir):
    """Split the tail-drain waits (same 1-wait-per-instruction limit)."""

    def _drain_and_barrier(self, tick_clock, wait_clock):
        drain_inst = self.nc.sync.drain()
        wait_clock.add_sem_waits(
            drain_inst.ins, bass_rust.ScopedClock({None: tick_clock.global_clock})
        )
        si = drain_inst.ins.sync_info
        waits = list(si.on_wait) if si is not None else []
        if len(waits) > 1:
            si.on_wait = waits[:1]
            for i in range(1, len(waits)):
                extra = self.nc.sync.drain()
                esi = extra.ins.sync_info
                if esi is None:
                    extra.ins.sync_info = mybir.SyncInfo(
                        on_wait=[waits[i]], on_update=[]
                    )
                else:
                    esi.on_wait = [waits[i]]
        self.nc.all_engine_barrier()
        assert self.sems is not None
        popped = self.nc._tile_sem_poison_stack.pop()
        assert popped is self._sem_poison
        sems = sorted(
            s.num if hasattr(s, "num") else s
            for s in self.sems.allocated().values()
        )
        for i in range(0, len(sems), 4):
            self.nc.clear_and_free_semaphores(sems[i : i + 4])
        self.nc.all_engine_barrier()

    tile.TileContext._drain_and_barrier = _drain_and_barrier


def build_program(T, ocnk, split_num, split_den, sub_q7):
    """T: per-slab diag round counts; ocnk: per-slab overflow chunk counts.

    Diag scheme: points binned by (x0, x1); round j of slab a holds the
    j-th point of each (a, q) bin at partition q (=x1), so the matmul
    stationary is a fixed identity and ONE fused tensor_scalar builds the
    value-scaled x2 one-hot. Bins deeper than T[a] overflow into generic
    two-one-hot chunks (the v1 path) appended to the same PSUM chain.
    """
    import os
    import concourse.bass as bass
    import concourse.mybir as mybir
    import concourse.tile as tile
    import bass_rust

    _patch_tile_drain(tile, bass_rust, mybir)

    f32 = mybir.dt.float32
    bf16 = mybir.dt.bfloat16
    Alu = mybir.AluOpType
    Act = mybir.ActivationFunctionType

    R = sum(T)
    ro = [0] * 128
    for a in range(1, 128):
        ro[a] = ro[a - 1] + T[a - 1]
    CO = max(1, sum(ocnk))
    oofs = [0] * 128
    for a in range(1, 128):
        oofs[a] = oofs[a - 1] + ocnk[a - 1]

    nc = bass.Bass("TRN2", target_bir_lowering=False, debug=False)
    x2r_d = nc.dram_tensor("x2r", [P, R], f32, kind="ExternalInput")
    vr_d = nc.dram_tensor("vr", [P, R], f32, kind="ExternalInput")
    x1o_d = nc.dram_tensor("x1o", [P, CO], f32, kind="ExternalInput")
    x2o_d = nc.dram_tensor("x2o", [P, CO], f32, kind="ExternalInput")
    vo_d = nc.dram_tensor("vo", [P, CO], f32, kind="ExternalInput")
    iota_d = nc.dram_tensor("iota", [P, P], bf16, kind="ExternalInput")
    ident_d = nc.dram_tensor("ident", [P, P], bf16, kind="ExternalInput")
    sdiff_d = nc.dram_tensor("sdiff", [P, P], f32, kind="ExternalInput")
    out_d = nc.dram_tensor("out", [1, 2], f32, kind="ExternalOutput")

    NACC = 48  # 32 d2-group slots + 8 d1 + 8 d3

    with tile.TileContext(nc) as tc:
        with tc.tile_pool(name="setup", bufs=1) as sp:
            iota_t = sp.tile([P, P], bf16)
            ident_t = sp.tile([P, P], bf16)
            sdiff_t = sp.tile([P, P], f32)
            nc.sync.dma_start(out=iota_t[:], in_=iota_d.ap()[:])
            nc.sync.dma_start(out=ident_t[:], in_=ident_d.ap()[:])
            nc.sync.dma_start(out=sdiff_t[:], in_=sdiff_d.ap()[:])
            x2r_t = sp.tile([P, R], f32)
            vr_t = sp.tile([P, R], f32)
            nch = 8
            for c0 in range(0, R, -(-R // nch)):
                c1 = min(R, c0 + -(-R // nch))
                nc.sync.dma_start(out=x2r_t[:, c0:c1], in_=x2r_d.ap()[:, c0:c1])
                nc.sync.dma_start(out=vr_t[:, c0:c1], in_=vr_d.ap()[:, c0:c1])
            x1o_t = sp.tile([P, CO], f32)
            x2o_t = sp.tile([P, CO], f32)
            vo_t = sp.tile([P, CO], f32)
            nc.sync.dma_start(out=x1o_t[:], in_=x1o_d.ap()[:])
            nc.sync.dma_start(out=x2o_t[:], in_=x2o_d.ap()[:])
            nc.sync.dma_start(out=vo_t[:], in_=vo_d.ap()[:])

            G = sp.tile([P, 128, XS], f32)  # [x1, x0, x2] grid
            accT = sp.tile([P, NACC], f32)
            accM = sp.tile([P, NACC], f32)
            nc.vector.memset(accT[:], 0.0)
            nc.vector.memset(accM[:], 0.0)

            with tc.tile_pool(name="oh", bufs=16) as oh, \
                 tc.tile_pool(name="ps", bufs=4, space="PSUM") as ps, \
                 tc.tile_pool(name="d2ps", bufs=2, space="PSUM") as d2ps, \
                 tc.tile_pool(name="fin", bufs=1, space="PSUM") as fin, \
                 tc.tile_pool(name="ls", bufs=2) as lsp:

                slot = [0]

                def emit_act_reduce(src_ap, nfd, wfd):
                    """abs-sum and square-sum of src ([P, nfd, wfd]) into the
                    next acc slot."""
                    n = slot[0]
                    dump = lsp.tile([P, 16, XS], f32, tag="dump")
                    nc.scalar.activation(
                        out=dump[:, :nfd, :wfd], in_=src_ap, func=Act.Abs,
                        accum_out=accT[:, n : n + 1],
                    )
                    dump2 = lsp.tile([P, 16, XS], f32, tag="dump2")
                    nc.scalar.activation(
                        out=dump2[:, :nfd, :wfd], in_=src_ap, func=Act.Square,
                        accum_out=accM[:, n : n + 1],
                    )
                    slot[0] += 1

                # ---- scatter: one PSUM accumulation chain per x0-slab ----
                ts_idx = 0
                for a in range(128):
                    nmm = T[a] + ocnk[a]
                    mi = 0
                    pt = ps.tile([P, 512], f32, space="PSUM", tag="scat")
                    for j in range(T[a]):
                        col = ro[a] + j
                        dm = oh.tile([P, P], bf16, tag="dm")
                        ts_idx += 1
                        if split_den and (ts_idx % split_den) < split_num:
                            # is_eq on DVE (2x mode), value-scale on Act
                            d01 = oh.tile([P, P], bf16, tag="d01")
                            nc.vector.tensor_scalar(
                                out=d01[:], in0=iota_t[:],
                                scalar1=x2r_t[:, col : col + 1],
                                scalar2=None, op0=Alu.is_equal,
                            )
                            nc.scalar.activation(
                                out=dm[:], in_=d01[:],
                                func=Act.Copy,
                                scale=vr_t[:, col : col + 1],
                            )
                        else:
                            nc.vector.tensor_scalar(
                                out=dm[:], in0=iota_t[:],
                                scalar1=x2r_t[:, col : col + 1],
                                scalar2=vr_t[:, col : col + 1],
                                op0=Alu.is_equal, op1=Alu.mult,
                            )
                        nc.tensor.matmul(
                            out=pt[:, :P], lhsT=ident_t[:], rhs=dm[:],
                            start=(mi == 0), stop=(mi == nmm - 1),
                        )
                        mi += 1
                    for j in range(ocnk[a]):
                        col = oofs[a] + j
                        bv = oh.tile([P, P], bf16, tag="bv")
                        nc.vector.tensor_scalar(
                            out=bv[:], in0=iota_t[:],
                            scalar1=x1o_t[:, col : col + 1],
                            scalar2=vo_t[:, col : col + 1],
                            op0=Alu.is_equal, op1=Alu.mult,
                        )
                        cm = oh.tile([P, P], bf16, tag="cm")
                        nc.vector.tensor_scalar(
                            out=cm[:], in0=iota_t[:],
                            scalar1=x2o_t[:, col : col + 1],
                            scalar2=None, op0=Alu.is_equal,
                        )
                        nc.tensor.matmul(
                            out=pt[:, :P], lhsT=bv[:], rhs=cm[:],
                            start=(mi == 0), stop=(mi == nmm - 1),
                        )
                        mi += 1
                    nc.scalar.copy(out=G[:, a, :], in_=pt[:, :P])

                    # d2 (x1/partition diffs) for each finished 4-slab group
                    if a % 4 == 3:
                        q = a // 4
                        dp = d2ps.tile([P, 4, XS], f32, space="PSUM", tag="d2")
                        nc.tensor.matmul(
                            out=dp[:], lhsT=sdiff_t[:],
                            rhs=G[:, 4 * q : 4 * q + 4, :],
                            start=True, stop=True,
                        )
                        emit_act_reduce(dp[:], 4, XS)

                    # d3 (x2 diffs) for each finished 16-slab group
                    sub_eng = nc.gpsimd if sub_q7 else nc.vector
                    if a % 16 == 15:
                        g = a // 16
                        lo = 16 * g
                        d = lsp.tile([P, 16, XS], f32, tag="dsub")
                        sub_eng.tensor_tensor(
                            out=d[:, :, : XS - 1],
                            in0=G[:, lo : lo + 16, 1:],
                            in1=G[:, lo : lo + 16, : XS - 1],
                            op=Alu.subtract,
                        )
                        emit_act_reduce(d[:, :, : XS - 1], 16, XS - 1)
                    # d1 (x0 diffs): group g needs slabs up to 16g+16
                    if (a % 16 == 0 and a > 0) or a == 127:
                        g = (a - 1) // 16 if a < 127 else 7
                        lo = 16 * g
                        npair = 16 if g < 7 else 15
                        d = lsp.tile([P, 16, XS], f32, tag="dsub")
                        sub_eng.tensor_tensor(
                            out=d[:, :npair, :],
                            in0=G[:, lo + 1 : lo + 1 + npair, :],
                            in1=G[:, lo : lo + npair, :],
                            op=Alu.subtract,
                        )
                        emit_act_reduce(d[:, :npair, :], npair, XS)

                # ---- optional DVE-variant probe (diagnostic only) ----
                if int(os.environ.get("TRNK_PROBE", "0")):
                    with tc.tile_pool(name="probe", bufs=4) as pp:
                        iota_f32 = sp.tile([P, P], f32)
                        nc.scalar.copy(out=iota_f32[:], in_=iota_t[:])
                        s1 = x2r_t[:, 0:1]
                        s2 = vr_t[:, 0:1]
                        NREP = 256
                        for i in range(NREP):
                            o = pp.tile([P, P], bf16, tag="p1")
                            nc.vector.tensor_scalar(
                                out=o[:], in0=iota_t[:], scalar1=s1, scalar2=s2,
                                op0=Alu.is_equal, op1=Alu.mult)
                        for i in range(NREP):
                            o = pp.tile([P, P], bf16, tag="p2")
                            nc.vector.tensor_scalar(
                                out=o[:], in0=iota_t[:], scalar1=s1, scalar2=None,
                                op0=Alu.is_equal)
                        for i in range(NREP):
                            o = pp.tile([P, P], bf16, tag="p3")
                            nc.vector.tensor_scalar(
                                out=o[:], in0=iota_t[:], scalar1=s2, scalar2=None,
                                op0=Alu.mult)
                        for i in range(NREP):
                            o = pp.tile([P, P], bf16, tag="p4")
                            nc.vector.tensor_scalar(
                                out=o[:], in0=iota_t[:], scalar1=3.0, scalar2=None,
                                op0=Alu.mult)
                        for i in range(NREP):
                            o = pp.tile([P, P], f32, tag="p5")
                            nc.vector.tensor_scalar(
                                out=o[:], in0=iota_t[:], scalar1=s1, scalar2=s2,
                                op0=Alu.is_equal, op1=Alu.mult)
                        for i in range(NREP):
                            o = pp.tile([P, P], bf16, tag="p6")
                            nc.vector.tensor_scalar(
                                out=o[:], in0=iota_f32[:], scalar1=s1, scalar2=s2,
                                op0=Alu.is_equal, op1=Alu.mult)
                        for i in range(NREP):
                            o = pp.tile([P, P], bf16, tag="p7")
                            nc.vector.tensor_scalar(
                                out=o[:], in0=iota_t[:], scalar1=s1, scalar2=3.0,
                                op0=Alu.is_equal, op1=Alu.mult)
                        for i in range(NREP):
                            o = pp.tile([P, P], bf16, tag="p8")
                            nc.vector.tensor_tensor(
                                out=o[:], in0=iota_t[:], in1=ident_t[:],
                                op=Alu.is_equal)
                        for i in range(NREP):
                            o = pp.tile([P, 2, P], bf16, tag="p9")
                            nc.vector.tensor_scalar(
                                out=o[:], in0=G[:, 0:2, :], scalar1=s1, scalar2=s2,
                                op0=Alu.is_equal, op1=Alu.mult)
                        for i in range(NREP):
                            o = pp.tile([P, P], bf16, tag="p10")
                            nc.scalar.activation(
                                out=o[:], in_=iota_t[:], func=Act.Copy, scale=s2)

                # ---- final: reduce slots, cross-partition sum, scale ----
                tvcol = sp.tile([P, 1], f32)
                msecol = sp.tile([P, 1], f32)
                nc.vector.tensor_reduce(
                    out=tvcol[:], in_=accT[:], axis=mybir.AxisListType.X,
                    op=Alu.add,
                )
                nc.vector.tensor_reduce(
                    out=msecol[:], in_=accM[:], axis=mybir.AxisListType.X,
                    op=Alu.add,
                )
                ones = sp.tile([P, 1], f32)
                nc.vector.memset(ones[:], 1.0)
                tv_ps = fin.tile([P, 512], f32, space="PSUM", tag="fint")
                nc.tensor.matmul(out=tv_ps[0:1, 0:1], lhsT=tvcol[:], rhs=ones[:],
                                 start=True, stop=True)
                mse_ps = fin.tile([P, 512], f32, space="PSUM", tag="finm")
                nc.tensor.matmul(out=mse_ps[0:1, 0:1], lhsT=msecol[:], rhs=ones[:],
                                 start=True, stop=True)
                res = sp.tile([1, 2], f32)
                nc.scalar.mul(out=res[:, 0:1], in_=tv_ps[0:1, 0:1], mul=1.0 / TV_NORM)
                nc.scalar.mul(out=res[:, 1:2], in_=mse_ps[0:1, 0:1], mul=1.0 / MSE_NORM)
                nc.sync.dma_start(out=out_d.ap()[:], in_=res[:])

    _split_waits(nc, mybir)
    return nc


_PROG_CACHE = {}


def _get_program(T, ocnk, split_num, split_den, sub_q7):
    key = (tuple(T), tuple(ocnk), split_num, split_den, sub_q7)
    if key not in _PROG_CACHE:
        _PROG_CACHE[key] = build_program(
            tuple(T), tuple(ocnk), split_num, split_den, sub_q7
        )
    return _PROG_CACHE[key]


def _pack_inputs(indices, values, tcap):
    """Bin points by (x0, x1); rounds place the j-th point of bin (a, q)
    at partition q; bins deeper than T[a] overflow into v1-style chunks."""
    import ml_dtypes

    i0 = indices[:, :, 0].astype(np.int64)
    i1 = indices[:, :, 1].astype(np.int64)
    i2 = indices[:, :, 2]
    key = i0 * P + i1
    cnt2 = np.zeros((B, 128 * P), np.int64)
    for b in range(B):
        cnt2[b] = np.bincount(key[b], minlength=128 * P)
    mx2 = cnt2.max(axis=0).reshape(128, P)  # worst-core per-(a,q) bin size
    T = np.maximum(1, np.minimum(mx2.max(axis=1), tcap)).astype(np.int64)
    ro = np.zeros(128, np.int64)
    ro[1:] = np.cumsum(T)[:-1]
    R = int(T.sum())
    ovc = np.maximum(0, cnt2.reshape(B, 128, P) - T[None, :, None]).sum(axis=2)
    ocnk = (-(-ovc.max(axis=0) // P)).astype(np.int64)  # [128]
    oofs = np.zeros(128, np.int64)
    oofs[1:] = np.cumsum(ocnk)[:-1]
    CO = max(1, int(ocnk.sum()))

    iota_in = np.broadcast_to(np.arange(P, dtype=np.float32), (P, P)).astype(
        ml_dtypes.bfloat16
    )
    ident_in = np.eye(P, dtype=np.float32).astype(ml_dtypes.bfloat16)

    in_maps = []
    for b in range(B):
        order = np.argsort(key[b], kind="stable")
        ks = key[b][order]
        a_s = ks // P
        q_s = ks % P
        i2s = i2[b][order].astype(np.float32)
        vs = values[b][order]
        starts2 = np.zeros(128 * P + 1, np.int64)
        starts2[1:] = np.cumsum(cnt2[b])
        rank = np.arange(M, dtype=np.int64) - starts2[ks]
        dmask = rank < T[a_s]
        X2R = np.zeros((P, R), np.float32)
        VR = np.zeros((P, R), np.float32)
        X2R[q_s[dmask], ro[a_s[dmask]] + rank[dmask]] = i2s[dmask]
        VR[q_s[dmask], ro[a_s[dmask]] + rank[dmask]] = vs[dmask]
        X1O = np.zeros((P, CO), np.float32)
        X2O = np.zeros((P, CO), np.float32)
        VO = np.zeros((P, CO), np.float32)
        ov = ~dmask
        if ov.any():
            cs = np.cumsum(ov)
            slab_start = starts2[a_s * P]  # first point idx of this slab
            cs0 = np.zeros(M + 1, np.int64)
            cs0[1:] = cs
            ovrank = cs - 1 - cs0[slab_start]
            col = oofs[a_s[ov]] + ovrank[ov] // P
            part = ovrank[ov] % P
            X1O[part, col] = q_s[ov].astype(np.float32)
            X2O[part, col] = i2s[ov]
            VO[part, col] = vs[ov]
        in_maps.append(
            {
                "x2r": X2R,
                "vr": VR,
                "x1o": X1O,
                "x2o": X2O,
                "vo": VO,
                "iota": iota_in,
                "ident": ident_in,
                "sdiff": _sdiff_mat(),
            }
        )
    return T, ocnk, in_maps


def _sdiff_mat():
    # sdiff[x1, m] = +1 if x1==m+1 else -1 if x1==m (column 127 zeroed)
    sd = np.zeros((P, P), np.float32)
    for m in range(P - 1):
        sd[m + 1, m] = 1.0
        sd[m, m] = -1.0
    return sd


def kernel(indices, values, xsize):
    sys.path.insert(0, "/opt/trn_rl_repo")
    _install_ntff_hook()
    from concourse import bass_utils

    indices = np.asarray(indices, dtype=np.int32)
    values = np.asarray(values, dtype=np.float32)
    assert int(xsize) == XS
    assert indices.shape == (B, M, 3) and values.shape == (B, M)

    import os

    tcap = int(os.environ.get("TRNK_TCAP", "32"))
    split_num = int(os.environ.get("TRNK_SPLIT_NUM", "0"))
    split_den = int(os.environ.get("TRNK_SPLIT_DEN", "0"))
    sub_q7 = int(os.environ.get("TRNK_SUB_Q7", "1"))
    T, ocnk, in_maps = _pack_inputs(indices, values, tcap)
    nc = _get_program(T, ocnk, split_num, split_den, sub_q7)

    trace = bool(os.environ.get("TRNK_TRACE"))
    res = bass_utils.run_bass_kernel_spmd(
        nc, in_maps, core_ids=list(range(B)), trace=trace
    )
    if trace and res.exec_time_ns is not None:
        print(f"HW exec time: {res.exec_time_ns} ns")
    tv = np.array([res.results[b]["out"][0, 0] for b in range(B)], np.float32)
    mse = np.array([res.results[b]["out"][0, 1] for b in range(B)], np.float32)
    return np.stack([tv, mse]).astype(np.float32)


if __name__ == "__main__":
    rng = np.random.default_rng(0)
    idx = rng.integers(0, XS, (B, M, 3), dtype=np.int32)
    val = rng.standard_normal((B, M), dtype=np.float32)
    out = kernel(idx, val, XS)
    print(out)
